# revision 27
# baseline (speedup 1.0000x reference)
"""Trainium2 Bass kernel for nn_MessagePassingEdgeModel (v11).

Reference computation (per edge e):
    h   = concat(x_s[src[e]], x_t[tgt[e]], edge_attr[e], x_u)      # [256]
    z   = leaky_relu(h @ W1 + b1, 0.01)                            # [256]
    y   = z @ W2 + b2                                              # [64]
    out = y * rsqrt(mean(y*y) + eps) * gamma                       # [64]

Distribution: edges are assigned to the 8 cores by the (src-half, tgt-half)
class of their endpoints (class k served by cores {2k, 2k+1}), so each core
addresses 25000 rows of each node table and indices fit in int16.

v13 (this file, 873746 ns on HW) changes vs v10 (934406 ns):
 - evt_wait analysis showed the real bottlenecks are PE (690us net: 3658
   matmuls + 3720 LDWEIGHTS) and the DMA engines (~540us; 256B gather
   descriptors pay a 2x small-transfer latency penalty).  GpSimd desc-gen
   is NOT a bottleneck (18.6us net; v10's 5.8ns/desc theory was wait-time
   inside the instruction, mostly DMA ring drain at ~4 engines/queue).
 - hi halves of both gathers are packed by DVE strided copies into gc
   [128, 4096] ([xs_hi|xt_hi] per edge row); 8 PE transposes/tile (vs 16)
   feed ONE [128,1024] psum->SBUF copy (alternating ACT/DVE).  PE runs
   22 matmuls/tile (8 transp + 8 L1 + 4 L2 + 2 colsum).
 - xs gather packed: edges sorted by src; descriptors cover 2/4 consecutive
   node rows (hi-only table, 256B-aligned windows): quad descs (512B, 4
   edges), pair descs (256B, 2 edges), parity-split singles.  ~0.46
   descs/edge instead of 1.0.  xt stays 1 desc/edge (the gc column order is
   fixed by the xs matching; both tables cannot pack simultaneously).
   xt is split over 3 SWDGE queues to balance per-queue drain (helped 27us).
 - epilogue runs on [128, 512] tiles (both 512-edge segments stacked on
   partitions) - full-lane DVE/ACT; colsum lands in the same psum tile
   as y (psL 2x2 banks + psY 1x2 + psT 2x1 = 8 banks).

Measured dead ends (do not retry):
 - dma_start_transpose (XBAR) SBUF->SBUF at volume: ~319ns per 16x128 tile
   on real HW (cost model says 14ns) - a small-packet storm (162k packets)
   that saturates DMA engines and backpressures the gather rings.  Single
   correctness is fine (unit-tested PASS), perf is not.  v11 with it: 1103us.
 - moving sq to DVE + gb bufs=3: 893us (worse than 873).
 - fp8 anywhere on the value path: e4m3 adds ~2.4% rel err (cancellation
   sums get no sqrt-averaging) vs the 2e-2 gate with 4.4e-3 already used.
Remaining leads: engines all ~55-60% busy (dependency stalls; wall 874us vs
max engine 520us) - finer psum pipelining / reordering may close the gap;
per-queue gather drain is ~10us/group (floor ~330us if perfectly hidden).
"""

import numpy as np
import ml_dtypes

BF = ml_dtypes.bfloat16
P = 128
D = 64
MSG = 256
TILE_E = 1024
GROUP = 4096
TPG = GROUP // TILE_E
HALF = 25000
WSUP = HALF // 2          # 256B super-rows in the hi-only xs table
LEAKY = 0.01
N_CORES = 8
EPS = float(np.finfo(np.float32).eps)


DEBUG_DUMP = False


def build_nc(t_groups, Q, PR, SE, SO):
    """Q/PR/SE/SO: per-group xs descriptor counts (quads / pairs / singles
    even / singles odd), each a multiple of 128, 4Q + 2PR + SE + SO == GROUP.
    """
    import concourse.bacc as bacc
    import concourse.tile as tile
    from concourse import mybir
    from concourse.ap import AP

    assert Q % P == 0 and PR % P == 0 and SE % P == 0 and SO % P == 0
    assert 4 * Q + 2 * PR + SE + SO == GROUP

    f32 = mybir.dt.float32
    bf16 = mybir.dt.bfloat16
    i16 = mybir.dt.int16
    AF = mybir.ActivationFunctionType
    e_pad = t_groups * GROUP
    PS = PR + SE + SO
    Qb, PRb, SEb, SOb = Q // P, PR // P, SE // P, SO // P
    # gc block offsets (128-col blocks, 32 per group) per region
    B_Q = 0
    B_P = 4 * Qb
    B_SE = B_P + 2 * PRb
    B_SO = B_SE + SEb
    assert B_SO + SOb == GROUP // P

    nc = bacc.Bacc(None, target_bir_lowering=False, debug=False,
                   num_swdge_queues=4)

    xsq = nc.dram_tensor("xsq", [HALF + 4, D], bf16, kind="ExternalInput")
    xth = nc.dram_tensor("xth", [HALF, P], bf16, kind="ExternalInput")
    qidx = nc.dram_tensor("qidx", [t_groups, P, max(Q, 16) // 16], i16,
                          kind="ExternalInput")
    pidx = nc.dram_tensor("pidx", [t_groups, P, max(PS, 16) // 16], i16,
                          kind="ExternalInput")
    tidx = nc.dram_tensor("tidx", [t_groups, P, GROUP // 16], i16,
                          kind="ExternalInput")
    eaT = nc.dram_tensor("eaT", [D + 1, e_pad], bf16, kind="ExternalInput")
    wa = nc.dram_tensor("wa", [P, MSG], bf16, kind="ExternalInput")
    wb = nc.dram_tensor("wb", [D + 1, MSG], bf16, kind="ExternalInput")
    w2 = nc.dram_tensor("w2", [P, P], bf16, kind="ExternalInput")
    onesb = nc.dram_tensor("onesb", [P, P], bf16, kind="ExternalInput")
    identb = nc.dram_tensor("identb", [P, P], bf16, kind="ExternalInput")
    cstp = nc.dram_tensor("cstp", [P, 3], f32, kind="ExternalInput")
    outT = nc.dram_tensor("outT", [P, t_groups * GROUP // 2], bf16,
                          kind="ExternalOutput")
    if DEBUG_DUMP:
        dbg_gc = nc.dram_tensor("dbg_gc", [P, GROUP], bf16,
                                kind="ExternalOutput")
        dbg_at = nc.dram_tensor("dbg_at", [P, GROUP // P, P], bf16,
                                kind="ExternalOutput")
        dbg_z = nc.dram_tensor("dbg_z", [P, TILE_E], bf16,
                               kind="ExternalOutput")
        dbg_py = nc.dram_tensor("dbg_py", [P, TILE_E], mybir.dt.float32,
                                kind="ExternalOutput")
        dbg_gxq = nc.dram_tensor("dbg_gxq", [P, max(Q // P, 1), 2 * P],
                                 bf16, kind="ExternalOutput")
        dbg_yb = nc.dram_tensor("dbg_yb", [P, 512], bf16,
                                kind="ExternalOutput")
        dbg_sq = nc.dram_tensor("dbg_sq", [P, 512], bf16,
                                kind="ExternalOutput")
        dbg_rsq = nc.dram_tensor("dbg_rsq", [P, 512], bf16,
                                 kind="ExternalOutput")
        dbg_ot = nc.dram_tensor("dbg_ot", [P, 512], bf16,
                                kind="ExternalOutput")

    # overlapping gather source views: stride 128 elems (256B), windows of
    # 256 elems (quads: rows 2w..2w+3) / 128 elems (pairs+singles: 2w,2w+1)
    xs_quad_ap = AP(xsq, 0, [[P, WSUP - 1], [1, 2 * P]])
    xs_pair_ap = AP(xsq, 0, [[P, WSUP], [1, P]])

    with tile.TileContext(nc) as tc:
        with (
            nc.allow_low_precision(reason="bf16 matmul path"),
            tc.tile_pool(name="const", bufs=1) as cp,
            tc.tile_pool(name="gb", bufs=2) as gb,
            tc.tile_pool(name="tb", bufs=2) as tb,
            tc.tile_pool(name="zb", bufs=3) as zb,
            tc.tile_pool(name="eb", bufs=3) as eb,
            tc.tile_pool(name="psl", bufs=2, space="PSUM") as psL,
            tc.tile_pool(name="psy", bufs=1, space="PSUM") as psY,
            tc.tile_pool(name="pst", bufs=2, space="PSUM") as psT,
        ):
            wa_t = cp.tile([P, MSG], bf16)
            nc.sync.dma_start(wa_t[:], wa[:])
            wb_t = cp.tile([D + 1, MSG], bf16)
            nc.sync.dma_start(wb_t[:], wb[:])
            w2_t = cp.tile([P, P], bf16)
            nc.sync.dma_start(w2_t[:], w2[:])
            on_t = cp.tile([P, P], bf16)
            nc.sync.dma_start(on_t[:], onesb[:])
            identb_t = cp.tile([P, P], bf16)
            nc.sync.dma_start(identb_t[:], identb[:])
            cst_t = cp.tile([P, 3], f32)
            nc.sync.dma_start(cst_t[:], cstp[:])
            b2col = cst_t[:, 0:1]
            scl = cst_t[:, 1:2]
            bia = cst_t[:, 2:3]

            for g in range(t_groups):
                qit = gb.tile([P, max(Q, 16) // 16], i16, tag="qit")
                nc.sync.dma_start(qit[:], qidx[g])
                pit = gb.tile([P, max(PS, 16) // 16], i16, tag="pit")
                nc.sync.dma_start(pit[:], pidx[g])
                tit = gb.tile([P, GROUP // 16], i16, tag="tit")
                nc.sync.dma_start(tit[:], tidx[g])

                gxq = gb.tile([P, max(Qb, 1), 2 * P], bf16, tag="gxq")
                gxp = gb.tile([P, max(PRb + SEb + SOb, 1), P], bf16,
                              tag="gxp")
                gt = gb.tile([P, GROUP // P, P], bf16, tag="gt")
                if Q > 0:
                    nc.gpsimd.dma_gather(
                        out_ap=gxq[:, 0:Qb, :],
                        in_ap=xs_quad_ap,
                        idxs_ap=qit[:],
                        num_idxs=Q,
                        num_idxs_reg=Q,
                        elem_size=2 * P,
                        elem_step=P,
                        transpose=False,
                        single_packet=False,
                        queue_num=0,
                    )
                if PS > 0:
                    nc.gpsimd.dma_gather(
                        out_ap=gxp[:, 0:(PRb + SEb + SOb), :],
                        in_ap=xs_pair_ap,
                        idxs_ap=pit[:],
                        num_idxs=PS,
                        num_idxs_reg=PS,
                        elem_size=P,
                        elem_step=P,
                        transpose=False,
                        single_packet=False,
                        queue_num=1,
                    )
                # xt split across 3 queues (128-desc aligned) to balance
                # per-queue DMA-engine drain against q0 (quads) / q1 (ps)
                xt_splits = [(0, 1280, 0), (1280, 2688, 2), (2688, 4096, 3)]
                for a, b, qn in xt_splits:
                    nc.gpsimd.dma_gather(
                        out_ap=gt[:, a // P:b // P, :],
                        in_ap=xth[:],
                        idxs_ap=tit[:, a // 16:b // 16],
                        num_idxs=b - a,
                        num_idxs_reg=b - a,
                        elem_size=P,
                        transpose=False,
                        single_packet=False,
                        queue_num=qn,
                    )
                ea_t = gb.tile([D + 1, GROUP], bf16, tag="ea")
                nc.sync.dma_start(ea_t[:],
                                  eaT[:, g * GROUP:(g + 1) * GROUP])

                # pack hi halves into gc: block j (128 cols) = edge j's
                # [xs_hi | xt_hi] feature row
                gc = gb.tile([P, GROUP], bf16, tag="gc")
                gc4 = gc[:].rearrange("p (b n) -> p b n", n=P)
                if Q > 0:
                    nc.vector.tensor_copy(
                        gc4[:, B_Q:B_P, 0:D].rearrange(
                            "p (q f) n -> p q f n", f=4),
                        gxq[:, 0:Qb, :].rearrange(
                            "p q (f n) -> p q f n", n=D))
                if PR > 0:
                    nc.vector.tensor_copy(
                        gc4[:, B_P:B_SE, 0:D].rearrange(
                            "p (q f) n -> p q f n", f=2),
                        gxp[:, 0:PRb, :].rearrange(
                            "p q (f n) -> p q f n", n=D))
                if SE > 0:
                    nc.vector.tensor_copy(
                        gc4[:, B_SE:B_SO, 0:D],
                        gxp[:, PRb:PRb + SEb, 0:D])
                if SO > 0:
                    nc.vector.tensor_copy(
                        gc4[:, B_SO:B_SO + SOb, 0:D],
                        gxp[:, PRb + SEb:PRb + SEb + SOb, D:P])
                nc.vector.tensor_copy(gc4[:, :, D:P], gt[:, :, 0:D])

                if DEBUG_DUMP and g == 0:
                    nc.sync.dma_start(dbg_gc[:], gc[:])
                    nc.sync.dma_start(dbg_gxq[:], gxq[:])

                for ti in range(TPG):
                    toff = ti * TILE_E
                    gtile = g * TPG + ti
                    # PE transposes: 8 blocks of [128e, 128f] -> psum, then
                    # one [128, 1024] copy to SBUF (alternating ACT/DVE)
                    ptc = psT.tile([P, TILE_E], bf16, tag="pt")
                    for j in range(8):
                        nc.tensor.transpose(
                            out=ptc[:, j * P:(j + 1) * P],
                            in_=gc4[:, ti * 8 + j, :],
                            identity=identb_t[:])
                    at = tb.tile([P, TILE_E], bf16, tag="at")
                    if ti % 2 == 0:
                        nc.scalar.activation(at[:], ptc[:], AF.Copy)
                    else:
                        nc.vector.tensor_copy(at[:], ptc[:])
                    if DEBUG_DUMP and g == 0 and ti == 0:
                        nc.sync.dma_start(
                            dbg_at[:],
                            at[:].rearrange("p (b n) -> p b n", n=P))

                    zc = []
                    for c in range(2):
                        pl = psL.tile([P, TILE_E], f32, tag="l1")
                        for s in range(2):
                            sl = slice(s * 512, (s + 1) * 512)
                            rsl = slice(toff + s * 512, toff + (s + 1) * 512)
                            nc.tensor.matmul(
                                pl[:, sl],
                                lhsT=wa_t[:, c * P:(c + 1) * P],
                                rhs=at[:, sl],
                                start=True, stop=False)
                            nc.tensor.matmul(
                                pl[:, sl],
                                lhsT=wb_t[:, c * P:(c + 1) * P],
                                rhs=ea_t[:, rsl],
                                start=False, stop=True)
                        z = zb.tile([P, TILE_E], bf16, tag="z")
                        nc.scalar.activation(z[:], pl[:], AF.Prelu,
                                             bias=0.0, scale=1.0,
                                             alpha=LEAKY)
                        zc.append(z)
                        if DEBUG_DUMP and g == 0 and ti == 0 and c == 0:
                            nc.sync.dma_start(dbg_z[:], z[:])

                    # py: cols 0:512 = y (segs stacked on partitions),
                    #     cols 512:1024 = colsum(sq) replicated
                    py = psY.tile([P, TILE_E], f32, tag="y")
                    # chunk-major: the c0 matmuls only need z chunk 0, so L2
                    # starts while prelu(c1) is still running
                    for c in range(2):
                        for s in range(2):
                            nc.tensor.matmul(
                                py[s * D:(s + 1) * D, 0:512],
                                lhsT=w2_t[:, c * D:(c + 1) * D],
                                rhs=zc[c][:, s * 512:(s + 1) * 512],
                                start=(c == 0), stop=(c == 1),
                                skip_group_check=True)

                    yb = eb.tile([P, 512], bf16, tag="yb")
                    nc.vector.tensor_scalar_add(yb[:], py[:, 0:512], b2col)
                    sq = eb.tile([P, 512], bf16, tag="sq")
                    nc.scalar.activation(sq[:], yb[:], AF.Square)
                    # block-diag ones: one K=128 matmul sums partitions 0:64
                    # into rows 0:64 and 64:128 into rows 64:128
                    nc.tensor.matmul(
                        py[:, 512:1024],
                        lhsT=on_t[:],
                        rhs=sq[:],
                        start=True, stop=True)
                    if DEBUG_DUMP and g == 0 and ti == 0:
                        dbt = eb.tile([P, TILE_E], mybir.dt.float32,
                                      tag="dbt")
                        nc.vector.tensor_copy(dbt[:], py[:])
                        nc.sync.dma_start(dbg_py[:], dbt[:])
                    rsq = eb.tile([P, 512], bf16, tag="rsq")
                    nc.scalar.activation(rsq[:], py[:, 512:1024],
                                         AF.Abs_reciprocal_sqrt,
                                         bias=bia, scale=scl)
                    ot = eb.tile([P, 512], bf16, tag="ot")
                    nc.vector.scalar_tensor_tensor(
                        out=ot[:], in0=yb[:], scalar=1.0,
                        in1=rsq[:], op0=mybir.AluOpType.mult,
                        op1=mybir.AluOpType.mult)
                    nc.sync.dma_start(
                        outT[:, gtile * 512:(gtile + 1) * 512], ot[:])
                    if DEBUG_DUMP and g == 0 and ti == 0:
                        nc.sync.dma_start(dbg_yb[:], yb[:])
                        nc.sync.dma_start(dbg_sq[:], sq[:])
                        nc.sync.dma_start(dbg_rsq[:], rsq[:])
                        nc.sync.dma_start(dbg_ot[:], ot[:])

    if not nc.is_finalized():
        nc.finalize()
    return nc


def _pack_hilo_rows(x):
    """[rows, 64] f32 -> [rows, 128] bf16 (hi | lo) row layout."""
    x = np.asarray(x, np.float32)
    hi = x.astype(BF)
    lo = (x - hi.astype(np.float32)).astype(BF)
    return np.ascontiguousarray(np.concatenate([hi, lo], axis=1))


def _wrap16(v, n):
    """[m<=n] int array -> [128, n//16] int16 (wrapped, replicated 8x)."""
    assert n % 16 == 0
    buf = np.full(n, -1, np.int16)
    buf[:len(v)] = v
    w = buf.reshape(-1, 16).T
    return np.ascontiguousarray(np.tile(w, (8, 1)))


def prep_shared(x_u, W1, b1, W2, b2, gamma):
    W1 = np.asarray(W1, np.float32)
    W2 = np.asarray(W2, np.float32)
    b1p = (np.asarray(b1, np.float32)
           + np.asarray(x_u, np.float32) @ W1[192:256])
    gamma = np.asarray(gamma, np.float32)
    w2p = np.empty((P, P), np.float32)
    w2p[:, 0:D] = W2[0:P]
    w2p[:, D:P] = W2[P:MSG]
    cstp = np.zeros((P, 3), np.float32)
    b2f = np.asarray(b2, np.float32)
    cstp[0:D, 0] = b2f
    cstp[D:P, 0] = b2f
    scl1 = 1.0 / (D * gamma * gamma)
    bia1 = EPS / (gamma * gamma)
    cstp[0:D, 1] = scl1
    cstp[D:P, 1] = scl1
    cstp[0:D, 2] = bia1
    cstp[D:P, 2] = bia1
    return {
        "wa": np.ascontiguousarray(W1[0:P].astype(BF)),
        "wb": np.ascontiguousarray(
            np.concatenate([W1[P:P + D], b1p[None, :]], 0).astype(BF)),
        "w2": w2p.astype(BF),
        "onesb": np.kron(np.eye(2), np.ones((D, D))).astype(BF),
        "identb": np.eye(P, dtype=BF),
        "cstp": cstp,
    }


def match_descs(src_local, order):
    """Greedy aligned matching of edges (sorted by src) into quad/pair/
    single descriptors.

    Returns (quads, pairs, se, so): each is (desc_idx_array, edge_id_array)
    with edge ids into the ORIGINAL per-core edge list; quads' edges are
    [n,4], pairs' [n,2], singles' [n].
    """
    cnt = np.bincount(src_local, minlength=HALF + 4)[:HALF]
    starts = np.zeros(HALF + 1, np.int64)
    np.cumsum(cnt, out=starts[1:])
    # edges of row r: order[starts[r]:starts[r]+cnt[r]]

    c4 = cnt[0:HALF - 3].reshape(-1, 4) if False else None
    k = HALF // 4
    quad_n = np.min(cnt[:4 * k].reshape(k, 4), axis=1)        # quads per 4-block
    rem = cnt[:4 * k] - np.repeat(quad_n, 4)
    rem = np.concatenate([rem, cnt[4 * k:]])
    w = HALF // 2
    pair_n = np.min(rem[:2 * w].reshape(w, 2), axis=1)        # pairs per super
    rem2 = rem[:2 * w] - np.repeat(pair_n, 2)

    # quads: m-th quad of block k takes edge m of each of rows 4k..4k+3
    qk = np.repeat(np.arange(k), quad_n)                       # block ids
    qm = np.concatenate([np.arange(n) for n in quad_n]) if len(qk) else \
        np.zeros(0, np.int64)
    quad_edges = np.empty((len(qk), 4), np.int64)
    for j in range(4):
        quad_edges[:, j] = order[starts[4 * qk + j] + qm]
    quad_widx = (2 * qk).astype(np.int16)                      # super-row idx

    pw = np.repeat(np.arange(w), pair_n)
    pm = np.concatenate([np.arange(n) for n in pair_n]) if len(pw) else \
        np.zeros(0, np.int64)
    pair_edges = np.empty((len(pw), 2), np.int64)
    for j in range(2):
        pair_edges[:, j] = order[starts[2 * pw + j]
                                 + quad_n_used(quad_n, 2 * pw + j) + pm]
    pair_widx = pw.astype(np.int16)

    # singles: remaining edges per row
    used = np.zeros(HALF, np.int64)
    used[:4 * k] += np.repeat(quad_n, 4)
    used[:2 * w] += np.repeat(pair_n, 2)
    nres = cnt - used
    rows = np.repeat(np.arange(HALF), nres)
    rm = np.concatenate([np.arange(n) for n in nres]) if rows.size else \
        np.zeros(0, np.int64)
    sing_edges = order[starts[rows] + used[rows] + rm]
    even = (rows % 2) == 0
    se = ((rows[even] // 2).astype(np.int16), sing_edges[even])
    so = ((rows[~even] // 2).astype(np.int16), sing_edges[~even])
    return (quad_widx, quad_edges), (pair_widx, pair_edges), se, so


def quad_n_used(quad_n, row):
    """edges of `row` already consumed by quads."""
    k = len(quad_n)
    qr = row // 4
    out = np.zeros(len(row), np.int64)
    m = qr < k
    out[m] = quad_n[qr[m]]
    return out


def choose_sizes(nq, npr, nse, nso, G):
    """Pick per-group (Q, PR, SE, SO), multiples of 128, 4Q+2PR+SE+SO=4096,
    feasible against global pools with demotion (quad->2 pairs,
    pair->2 singles split by parity)."""
    best = None
    for Q in range(min(nq // G, GROUP // 4) // P * P, -1, -P):
        pool_p = npr + 2 * (nq - G * Q)
        maxPR = min(pool_p // G, (GROUP - 4 * Q) // 2)
        for PR in range(maxPR // P * P, -1, -P):
            rem = GROUP - 4 * Q - 2 * PR
            # singles needed per group = rem (split SE/SO); global singles
            # pool after demotions (demoted pairs yield 1 even + 1 odd):
            dp = pool_p - G * PR
            pse = nse + dp
            pso = nso + dp
            # choose SE multiple of 128, SO = rem - SE
            SE = max(min(int(round(pse / (pse + pso + 1e-9) * rem)) // P * P,
                         rem), 0)
            SO = rem - SE
            if SO < 0 or SE % P or SO % P:
                continue
            # dummies allowed, so any split works; accept
            best = (Q, PR, SE, SO)
            break
        if best:
            break
    assert best is not None
    return best


def prep_core(core, eids, src, tgt, ea, xs_qtab, xt_half, t_groups,
              sizes, shared):
    """Build per-core input map + slot permutation.

    eids: int64 edge ids for this core.  Returns (in_map, perm) where
    perm[slot] = original edge id or -1 for padding slots.
    """
    Q, PR, SE, SO = sizes
    k = core // 2
    hs, ht = k >> 1, k & 1
    e_pad = t_groups * GROUP

    src_l = (src[eids] - hs * HALF).astype(np.int64)
    tgt_l = (tgt[eids] - ht * HALF).astype(np.int64)
    order = np.argsort(src_l, kind="stable")
    (qw, qe), (pw, pe), (sew, see), (sow, soe) = match_descs(src_l, order)

    # global desc pools; per group consume fixed counts, demoting as needed
    Qb, PRb, SEb, SOb = Q // P, PR // P, SE // P, SO // P
    B_P = 4 * Qb
    B_SE = B_P + 2 * PRb
    B_SO = B_SE + SEb

    qidx = np.zeros((t_groups, P, max(Q, 16) // 16), np.int16)
    pidx = np.zeros((t_groups, P, max(PR + SE + SO, 16) // 16), np.int16)
    tidx = np.zeros((t_groups, P, GROUP // 16), np.int16)
    perm = np.full(t_groups * GROUP, -1, np.int64)

    # pools as lists we pop from; demote when short
    quads_w = list(qw[::-1]); quads_e = list(qe[::-1])
    pairs_w = list(pw[::-1]); pairs_e = list(pe[::-1])
    se_w = list(sew[::-1]); se_e = list(see[::-1])
    so_w = list(sow[::-1]); so_e = list(soe[::-1])

    def pop_quad():
        if quads_w:
            return quads_w.pop(), quads_e.pop()
        return None

    def pop_pair():
        if pairs_w:
            return pairs_w.pop(), pairs_e.pop()
        q = pop_quad()
        if q is not None:
            w, e4 = q
            pairs_w.append(w + 1); pairs_e.append(e4[2:4])
            return w, e4[0:2]
        return None

    def pop_single(par):
        ww, ee = (se_w, se_e) if par == 0 else (so_w, so_e)
        if ww:
            return ww.pop(), ee.pop()
        p2 = pop_pair()
        if p2 is not None:
            w, e2 = p2
            # pair rows (2w, 2w+1): even sub -> se pool, odd -> so pool
            se_w.append(w); se_e.append(e2[0])
            so_w.append(w); so_e.append(e2[1])
            return pop_single(par)
        return None

    for g in range(t_groups):
        gbase = g * GROUP
        qarr = np.zeros(max(Q, 16), np.int16)
        for d in range(Q):
            item = pop_quad()
            pp, bb = d % P, d // P
            if item is not None:
                w, e4 = item
                qarr[d] = w
                for s in range(4):
                    perm[gbase + (4 * bb + s) * P + pp] = e4[s]
            else:
                qarr[d] = 0
        parr = np.zeros(max(PR + SE + SO, 16), np.int16)
        for d in range(PR):
            item = pop_pair()
            pp, bb = d % P, d // P
            if item is not None:
                w, e2 = item
                parr[d] = w
                for s in range(2):
                    perm[gbase + (B_P + 2 * bb + s) * P + pp] = e2[s]
            else:
                parr[d] = 0
        for d in range(SE):
            item = pop_single(0)
            pp, bb = d % P, d // P
            if item is not None:
                w, e1 = item
                parr[PR + d] = w
                perm[gbase + (B_SE + bb) * P + pp] = e1
            else:
                parr[PR + d] = 0
        for d in range(SO):
            item = pop_single(1)
            pp, bb = d % P, d // P
            if item is not None:
                w, e1 = item
                parr[PR + SE + d] = w
                perm[gbase + (B_SO + bb) * P + pp] = e1
            else:
                parr[PR + SE + d] = 0
        qidx[g] = _wrap16(qarr[:Q], max(Q, 16))
        pidx[g] = _wrap16(parr[:PR + SE + SO], max(PR + SE + SO, 16))
        gperm = perm[gbase:gbase + GROUP]
        valid = gperm >= 0
        tv = np.zeros(GROUP, np.int64)
        tv[valid] = tgt_l[gperm[valid]]
        tidx[g] = _wrap16(tv.astype(np.int16), GROUP)

    # leftover pools must be empty (capacity >= edges)
    assert not quads_w and not pairs_w and not se_w and not so_w, (
        len(quads_w), len(pairs_w), len(se_w), len(so_w))

    valid = perm >= 0
    ea_r = np.zeros((e_pad, D), np.float32)
    ea_r[valid] = ea[eids[perm[valid]]]
    eaT = np.empty((D + 1, e_pad), BF)
    eaT[0:D] = ea_r.T.astype(BF)
    eaT[D] = BF(1.0)

    return {"xsq": xs_qtab[hs], "xth": xt_half[ht],
            "qidx": qidx, "pidx": pidx, "tidx": tidx, "eaT": eaT,
            **shared}, perm


def assign_edges(src, tgt):
    """Split edges into 8 per-core id lists by (src-half, tgt-half) class."""
    cls = (src >= HALF).astype(np.int64) * 2 + (tgt >= HALF)
    order = np.argsort(cls, kind="stable")
    counts = np.bincount(cls, minlength=4)
    lists = []
    pos = 0
    for kk in range(4):
        chunk = order[pos:pos + counts[kk]]
        pos += counts[kk]
        n0 = (len(chunk) + 1) // 2
        lists.append(chunk[:n0])
        lists.append(chunk[n0:])
    return lists


_CACHE = {}
TRACE = False
LAST_RESULT = None


def kernel(x_s, x_t, edge_index, edge_attr, x_u, W1, b1, W2, b2, gamma):
    global LAST_RESULT
    from concourse.bass_utils import run_bass_kernel_spmd

    src = np.asarray(edge_index[0], np.int64)
    tgt = np.asarray(edge_index[1], np.int64)
    ea = np.asarray(edge_attr, np.float32)
    x_s = np.asarray(x_s, np.float32)
    x_t = np.asarray(x_t, np.float32)
    e_total = src.shape[0]

    lists = assign_edges(src, tgt)
    n_max = max(len(l) for l in lists)
    t_groups = -(-n_max // GROUP)

    # per-core matching stats to choose one shared (Q, PR, SE, SO)
    shared = prep_shared(x_u, W1, b1, W2, b2, gamma)
    xs_qtab = []
    for h in range(2):
        tab = np.zeros((HALF + 4, D), BF)
        tab[:HALF] = x_s[h * HALF:(h + 1) * HALF].astype(BF)
        xs_qtab.append(np.ascontiguousarray(tab))
    xt_half = [_pack_hilo_rows(x_t[0:HALF]), _pack_hilo_rows(x_t[HALF:])]

    # worst-case (most conservative) sizes across cores
    sizes = None
    for c in range(N_CORES):
        eids = lists[c]
        k = c // 2
        hs = k >> 1
        src_l = (src[eids] - hs * HALF).astype(np.int64)
        order = np.argsort(src_l, kind="stable")
        (qw, qe), (pw, pe), (sew, see), (sow, soe) = \
            match_descs(src_l, order)
        s = choose_sizes(len(qw), len(pw), len(sew), len(sow), t_groups)
        if sizes is None:
            sizes = s
        else:
            # feasible for all cores: take min Q, then min PR at that Q
            sizes = (min(sizes[0], s[0]), min(sizes[1], s[1]),
                     sizes[2], sizes[3])
    Q, PR = sizes[0], sizes[1]
    rem = GROUP - 4 * Q - 2 * PR
    SE = (rem // 2) // P * P
    SO = rem - SE
    sizes = (Q, PR, SE, SO)

    key = (t_groups, *sizes, DEBUG_DUMP)
    if key not in _CACHE:
        _CACHE[key] = build_nc(t_groups, *sizes)
    nc = _CACHE[key]

    in_maps = []
    perms = []
    for c in range(N_CORES):
        m, perm = prep_core(c, lists[c], src, tgt, ea, xs_qtab, xt_half,
                            t_groups, sizes, shared)
        in_maps.append(m)
        perms.append(perm)

    res = run_bass_kernel_spmd(nc, in_maps, list(range(N_CORES)), trace=TRACE)
    LAST_RESULT = res

    out = np.empty((e_total, D), np.float32)
    n_tiles = t_groups * TPG
    for c in range(N_CORES):
        perm = perms[c]
        valid = perm >= 0
        o = res.results[c]["outT"].astype(np.float32)      # [128, nT*512]
        o = o.reshape(2, D, n_tiles, 512).transpose(2, 0, 3, 1)
        o = o.reshape(n_tiles * TILE_E, D)                 # slot-major
        out[lists[c][perm[valid]]] = o[valid]              # perm is core-local
    return out
